# revision 48
# baseline (speedup 1.0000x reference)
"""Trainium2 Bass kernel for nn_Attention_9560597201123.

Full multi-head attention (B=4, N=2048, E=1024, H=16, D=64), f32 reference.

Sharding: 8 cores = (batch b in 0..4) x (sequence half in 0..2). Each core:
  - receives x[b].T (query-half columns first), full qkv/proj weights
  - computes k/v projections for the FULL batch-b sequence (2x redundant vs
    an exchange, but avoids slow 2-rank collectives entirely)
  - computes q projection + attention + output projection for its 1024
    query rows, returns y [1024, 1024]
Host assembles the 8 shards into [4, 2048, 1024].

Device layout notes:
  - scores are computed transposed (S^T: keys on partitions, queries free)
    so that P^T = exp(S^T) feeds the PV matmul directly (contraction = keys)
  - softmax normalizer: v is extended with a ones column (lhsT M=65), so
    the PV matmul's partition 64 accumulates the row sums for free
  - all TensorEngine matmuls run in bf16 (1 cycle/row); accumulation f32
  - chunked startup DMAs on parallel queues (xt per-ec on sync; weights on
    gpsimd/scalar) so the first matmuls start ~17us in
  - V projection interleaved with attention-0 qb=0 so the ScalarE exp
    stream (the per-phase floor: 256 x ~1.1us, one elem/cycle/lane) starts
    during the V phase
  - K/Q projections of pair p+1 emitted as small quanta between the S and
    PV matmuls of pair p's kc loop: TensorE's queue is strict FIFO, so the
    quanta must sit between S and PV to cover PV's exp-wait
  - k/q evictions on VectorE (tensor_scalar add) keeping ScalarE pure exp
  - softmax 1/Z via one partition-batched reciprocal_approx_fast per flush
    (rows gathered to partitions 0/32/64/96; free-dim cost is partition
    -count-independent) instead of 32 serial [1,512] reciprocals
  - tail: att(7,1) emitted before any output projection (FIFO head-of-line)
    and the projection split in halves so its first half overlaps the last
    exp stream and the last norm chain overlaps the first half
  - HAM warm-up: two short dummy-matmul bursts (gated on x chunks 0 and 4)
    keep the PE activity monitor at K=8/8 through the DMA-gated startup;
    without them the clock gate re-throttles to 1.2 GHz until ~36us
"""

import numpy as np
import ml_dtypes

P = 128
SEQ = 2048
QH = 1024  # queries per core
E = 1024
H = 16
NPAIR = 8  # head pairs
D = 64
KC = 16  # key chunks of 128
EC = 8  # e_in chunks of 128
SCALE = D ** -0.5  # 0.125

_NC = None


def build_nc():
    global _NC
    if _NC is not None:
        return _NC

    import concourse.bass as bass  # noqa: F401
    import concourse.mybir as mybir
    import concourse.tile as tile
    from concourse import bacc

    BF = mybir.dt.bfloat16
    F32 = mybir.dt.float32
    EXP = mybir.ActivationFunctionType.Exp
    ADD = mybir.AluOpType.add
    MULT = mybir.AluOpType.mult

    nc = bacc.Bacc("TRN2", target_bir_lowering=False, debug=False, num_devices=8)

    xt_d = nc.dram_tensor("xt", [E, SEQ], BF, kind="ExternalInput").ap()
    wqkv_d = nc.dram_tensor("wqkv", [E, 3 * E], BF, kind="ExternalInput").ap()
    bqkv_d = nc.dram_tensor("bqkv", [3 * E], F32, kind="ExternalInput").ap()
    wp_d = nc.dram_tensor("wp", [E, E], BF, kind="ExternalInput").ap()
    bv_bf_d = nc.dram_tensor("bv_bf", [E], BF, kind="ExternalInput").ap()
    bp_bf_d = nc.dram_tensor("bp_bf", [E], BF, kind="ExternalInput").ap()
    out_d = nc.dram_tensor("out", [QH, E], F32, kind="ExternalOutput").ap()

    wqkv_r = wqkv_d.rearrange("(o p) c -> p o c", p=P)

    with tile.TileContext(nc) as tc:
        with (
            tc.tile_pool(name="persist", bufs=1) as persist,
            tc.tile_pool(name="wstream", bufs=2) as wstream,
            tc.tile_pool(name="ptpool", bufs=4) as ptpool,
            tc.tile_pool(name="asbp", bufs=5) as asbp,
            tc.tile_pool(name="small", bufs=2) as small,
            tc.tile_pool(name="normp", bufs=1) as normp,
            tc.tile_pool(name="proj_ps", bufs=2, space="PSUM") as proj_ps,
            tc.tile_pool(name="acc_ps", bufs=2, space="PSUM") as acc_ps,
            tc.tile_pool(name="sc_ps", bufs=2, space="PSUM") as sc_ps,
        ):
            # ---- persistent tiles + input DMA ----
            # x lands in chunks on the sync queue so the first K/Q matmuls
            # start as early as possible
            xt_r = xt_d.rearrange("(o p) s -> p o s", p=P)
            xt3 = persist.tile([P, EC, SEQ], BF, tag="xt")
            for c in range(EC):
                nc.sync.dma_start(xt3[:, c : c + 1, :], xt_r[:, c : c + 1, :])
            xt = [xt3[:, ec, :] for ec in range(EC)]

            # HAM warm-up: ~4us of dummy matmuls gated only on xt chunk 0.
            # They fill the startup DMA wait and push the PE clock-gate to
            # K=8/8 before the real (DMA-gated, gappy) projection stream
            # begins; without this the PE re-throttles to 1.2 GHz until
            # ~36us and early matmuls run at 2x duration.
            warm = proj_ps.tile([P, 512], mybir.dt.float32, tag="ps512")
            for i in range(14):
                nc.tensor.matmul(
                    warm[:, 0:256],
                    lhsT=xt[0][:, 0:128],
                    rhs=xt[0][:, 0:256],
                    start=True,
                    stop=True,
                )
            # second mini-burst gated on a mid-stream chunk keeps the PE
            # busy across the remaining x-chunk DMA window (a >3.4us gap
            # would re-throttle the clock gate)
            for i in range(12):
                nc.tensor.matmul(
                    warm[:, 0:256],
                    lhsT=xt[4][:, 0:128],
                    rhs=xt[4][:, 0:256],
                    start=True,
                    stop=True,
                )

            vx4 = []
            for sm in range(KC):
                t = persist.tile([P, H * 65], BF, tag=f"vx{sm}", name=f"vx{sm}").rearrange(
                    "p (h c) -> p h c", c=65
                )
                nc.vector.memset(t[:, :, 64], 1.0)
                vx4.append(t)

            kt = [persist.tile([P, SEQ], BF, tag=f"kt{p}", name=f"kt{p}") for p in range(NPAIR)]
            qt = [persist.tile([P, QH], BF, tag=f"qt{p}", name=f"qt{p}") for p in range(NPAIR)]
            aT = [persist.tile([P, QH], BF, tag=f"aT{p}", name=f"aT{p}") for p in range(NPAIR)]

            bq_k = persist.tile([P, EC], F32, tag="bq_k")
            bq_q = persist.tile([P, EC], F32, tag="bq_q")

            bv_row = persist.tile([1, E], BF, tag="bv_row")
            nc.gpsimd.dma_start(bv_row[:], bv_bf_d[None])
            bv_bc = persist.tile([P, E], BF, tag="bv_bc")
            nc.gpsimd.partition_broadcast(bv_bc[:], bv_row[:])

            pb_row = persist.tile([1, E], BF, tag="pb_row")
            nc.gpsimd.dma_start(pb_row[:], bp_bf_d[None])
            pb_bc = persist.tile([P, E], BF, tag="pb_bc")
            nc.gpsimd.partition_broadcast(pb_bc[:], pb_row[:])

            pw = None  # allocated after the wvpool frees its SBUF space

            # ---- emission helpers ----

            def kq_quanta(p):
                """Return a list of emission closures for pair p's K/Q
                projections: 6 output blocks x 2 half-blocks (4 matmuls
                each). Interleaved into the attention kc loop so the exp
                stream never starves behind a long projection burst."""
                wq = wstream.tile([P, EC, P], BF, tag="wq")
                nc.gpsimd.dma_start(wq[:], wqkv_r[:, :, p * P : (p + 1) * P])
                wk = wstream.tile([P, EC, P], BF, tag="wk")
                nc.gpsimd.dma_start(wk[:], wqkv_r[:, :, E + p * P : E + (p + 1) * P])
                blocks = [("q", s) for s in range(2)] + [("k", s) for s in range(4)]
                quanta = []
                state = {}

                def mk(which, s, half):
                    w = wq if which == "q" else wk
                    dst = qt[p] if which == "q" else kt[p]
                    bias = bq_q if which == "q" else bq_k

                    def q_():
                        if half == 0:
                            state[(which, s)] = proj_ps.tile(
                                [P, 512],
                                mybir.dt.float32,
                                tag="ps512",
                                name=f"ps_{p}_{which}{s}",
                            )
                        ps = state[(which, s)]
                        for ec in range(4 * half, 4 * half + 4):
                            nc.tensor.matmul(
                                ps[:],
                                lhsT=w[:, ec, :],
                                rhs=xt[ec][:, s * 512 : (s + 1) * 512],
                                start=(ec == 0),
                                stop=(ec == EC - 1),
                            )
                        if half == 1:
                            nc.vector.tensor_scalar(
                                out=dst[:, s * 512 : (s + 1) * 512],
                                in0=ps[:],
                                scalar1=bias[:, p : p + 1],
                                scalar2=None,
                                op0=ADD,
                            )

                    return q_

                for which, s in blocks:
                    quanta.append(mk(which, s, 0))
                    quanta.append(mk(which, s, 1))
                return quanta

            def emit_kq(p):
                for q_ in kq_quanta(p):
                    q_()

            def emit_v_chunk(wv_tiles, sm):
                # both vc halves of v for key chunk sm (completes vx4[sm])
                for vc in range(2):
                    ps = proj_ps.tile([P, 512], mybir.dt.float32, tag="ps512")
                    for ec in range(EC):
                        nc.tensor.matmul(
                            ps[:],
                            lhsT=xt[ec][:, sm * P : (sm + 1) * P],
                            rhs=wv_tiles[vc][:, ec, :],
                            start=(ec == 0),
                            stop=(ec == EC - 1),
                        )
                    nc.vector.tensor_tensor(
                        out=vx4[sm][:, vc * 8 : (vc + 1) * 8, 0:64],
                        in0=ps[:].rearrange("p (h c) -> p h c", c=64),
                        in1=bv_bc[:, vc * 512 : (vc + 1) * 512].rearrange(
                            "p (h c) -> p h c", c=64
                        ),
                        op=ADD,
                    )

            pending_rows = []  # (asb, hh, p, qsl) awaiting normalize

            def emit_S(p, qb, kc):
                qsl = slice(qb * 512, (qb + 1) * 512)
                sc = sc_ps.tile([P, 1024], mybir.dt.float32, tag="sc")
                nc.tensor.matmul(
                    sc[:, 0:512],
                    lhsT=kt[p][0:64, kc * P : (kc + 1) * P],
                    rhs=qt[p][0:64, qsl],
                )
                nc.tensor.matmul(
                    sc[:, 512:1024],
                    lhsT=kt[p][64:P, kc * P : (kc + 1) * P],
                    rhs=qt[p][64:P, qsl],
                )
                pt = ptpool.tile([P, 1024], BF, tag="pt")
                nc.scalar.activation(out=pt[:], in_=sc[:], func=EXP, scale=SCALE)
                return pt

            def emit_PV(p, kc, pt, accA, accB):
                nc.tensor.matmul(
                    accA[:],
                    lhsT=vx4[kc][:, 2 * p, :],
                    rhs=pt[:, 0:512],
                    start=(kc == 0),
                    stop=(kc == KC - 1),
                )
                nc.tensor.matmul(
                    accB[:],
                    lhsT=vx4[kc][:, 2 * p + 1, :],
                    rhs=pt[:, 512:1024],
                    start=(kc == 0),
                    stop=(kc == KC - 1),
                )

            def emit_att_kc(p, qb, kc, accA, accB):
                pt = emit_S(p, qb, kc)
                emit_PV(p, kc, pt, accA, accB)

            def evict_acc(p, qb, accA, accB):
                qsl = slice(qb * 512, (qb + 1) * 512)
                for hh, acc in ((0, accA), (1, accB)):
                    asb = asbp.tile([65, 512], F32, tag="asb")
                    nc.vector.tensor_copy(out=asb[:], in_=acc[:])
                    pending_rows.append((asb, hh, p, qsl))

            def emit_att_qb(p, qb, fill=None):
                """Attention for (p, qb) in 2-kc groups: both S-pairs (64-row
                array config) back to back, then 4 PV matmuls (128-row
                config), then fill quanta. Halves the tile-config switches
                whose LDWEIGHTS cannot be pulled ahead. `fill` is a mutable
                list of emission closures (projection quanta) drained two per
                group so TensorE fill-work stays fine-grained."""
                accA = acc_ps.tile([65, 512], mybir.dt.float32, tag="acc")
                accB = acc_ps.tile([65, 512], mybir.dt.float32, tag="acc")
                for g in range(KC // 2):
                    pt0 = emit_S(p, qb, 2 * g)
                    pt1 = emit_S(p, qb, 2 * g + 1)
                    # fill BETWEEN S and PV: TensorE's queue is strict FIFO,
                    # so work placed after PV cannot cover PV's exp-wait.
                    # One quantum per group spreads the 12 quanta across both
                    # qb windows, feeding qb1's otherwise exp-gated groups.
                    if fill:
                        fill.pop(0)()
                    emit_PV(p, 2 * g, pt0, accA, accB)
                    emit_PV(p, 2 * g + 1, pt1, accA, accB)
                evict_acc(p, qb, accA, accB)
                while fill:
                    fill.pop(0)()

            def flush_norm():
                """Normalize all pending rows (max 4) with one
                partition-batched reciprocal (free-dim cost is independent of
                partition count). Rows are spaced 32 partitions apart because
                DVE access patterns must start on a 32-aligned partition."""
                rows = list(pending_rows)
                pending_rows.clear()
                if not rows:
                    return
                assert len(rows) <= 4
                # hh=1 first: its chain ends in a DMA that then overlaps the
                # hh=0 multiplies
                rows.sort(key=lambda r: -r[1])
                zg = normp.tile([97, 512], F32, tag="zg")
                for i, (asb, hh, p, qsl) in enumerate(rows):
                    nc.sync.dma_start(zg[32 * i : 32 * i + 1, :], asb[64:65, :])
                zr = normp.tile([97, 512], F32, tag="zr")
                nc.vector.reciprocal_approx_fast(out=zr[:], in_=zg[:])
                for i, (asb, hh, p, qsl) in enumerate(rows):
                    rs = small.tile([1, 512], BF, tag="rs")
                    nc.vector.tensor_copy(out=rs[:], in_=zr[32 * i : 32 * i + 1, :])
                    R = small.tile([64, 512], BF, tag="R")
                    nc.gpsimd.partition_broadcast(R[:], rs[:])
                    if hh == 0:
                        nc.vector.tensor_tensor(
                            out=aT[p][0:64, qsl], in0=asb[0:64, :], in1=R[:], op=MULT
                        )
                    else:
                        tmpb = small.tile([64, 512], BF, tag="tmpb")
                        nc.vector.tensor_tensor(
                            out=tmpb[:], in0=asb[0:64, :], in1=R[:], op=MULT
                        )
                        nc.sync.dma_start(aT[p][64:P, qsl], tmpb[:])

            def emit_proj(pw, qc_list, dma_eng=None):
                for qc in qc_list:
                    for ncol in range(2):
                        nsl = slice(ncol * 512, (ncol + 1) * 512)
                        yps = proj_ps.tile([P, 512], mybir.dt.float32, tag="ps512")
                        for p in range(NPAIR):
                            nc.tensor.matmul(
                                yps[:],
                                lhsT=aT[p][:, qc * P : (qc + 1) * P],
                                rhs=pw[:, p, nsl],
                                start=(p == 0),
                                stop=(p == NPAIR - 1),
                            )
                        ysb = small.tile([P, 512], F32, tag="ysb")
                        nc.vector.tensor_tensor(
                            out=ysb[:], in0=yps[:], in1=pb_bc[:, nsl], op=ADD
                        )
                        (dma_eng or nc.sync).dma_start(
                            out_d[qc * P : (qc + 1) * P, nsl], ysb[:]
                        )

            # ---- schedule ----
            # pair 0 K/Q first; V interleaved with attention-0 qb=0 so the
            # exp stream starts during the V projection; then the usual
            # software pipeline (K/Q of pair p+1 ahead of attention p).
            q0 = kq_quanta(0)  # wq0/wk0 DMAs land first on the gpsimd queue
            nc.gpsimd.dma_start(bq_k[:], bqkv_d[E : 2 * E].rearrange("(o p) -> p o", p=P))
            nc.gpsimd.dma_start(bq_q[:], bqkv_d[0:E].rearrange("(o p) -> p o", p=P))
            for q_ in q0:
                q_()
            with tc.tile_pool(name="wvpool", bufs=2) as wvpool:
                wv_tiles = []
                for vc in range(2):
                    wv = wvpool.tile([P, EC, 512], BF, tag="wv")
                    nc.scalar.dma_start(
                        wv[:], wqkv_r[:, :, 2 * E + vc * 512 : 2 * E + (vc + 1) * 512]
                    )
                    wv_tiles.append(wv)
                accA = acc_ps.tile([65, 512], mybir.dt.float32, tag="acc")
                accB = acc_ps.tile([65, 512], mybir.dt.float32, tag="acc")
                for sm in range(KC):
                    emit_v_chunk(wv_tiles, sm)
                    emit_att_kc(0, 0, sm, accA, accB)
                evict_acc(0, 0, accA, accB)

            with tc.tile_pool(name="pwpool", bufs=1) as pwpool:
                # proj weights on the scalar DMA queue (after wv) so they
                # don't contend with the startup xt stream on sync
                pw = pwpool.tile([P, NPAIR, E], BF, tag="pw")
                nc.scalar.dma_start(pw[:], wp_d.rearrange("(o p) c -> p o c", p=P))

                fill = kq_quanta(1)
                emit_att_qb(0, 1, fill)
                for p in range(1, NPAIR - 1):
                    flush_norm()
                    fill = kq_quanta(p + 1)
                    emit_att_qb(p, 0, fill)
                    emit_att_qb(p, 1, fill)
                # last pair: att(7,1) is emitted BEFORE any proj so its exp
                # stream follows att(7,0) without a stall (TensorE queue is
                # strict FIFO — proj emitted earlier would head-of-line block
                # it behind the norm chain). proj qc 0-3 needs only qb=0
                # norms, which complete while att(7,1) runs; the (7,1) norm
                # chain then overlaps proj qc 0-3.
                flush_norm()
                emit_att_qb(NPAIR - 1, 0)
                flush_norm()
                emit_att_qb(NPAIR - 1, 1)
                emit_proj(pw, range(0, 4), dma_eng=nc.scalar)
                flush_norm()
                emit_proj(pw, range(4, 8))

    nc.finalize()
    _NC = nc
    return nc


def make_in_maps(x, qkv_w, qkv_b, proj_w, proj_b):
    bf16 = ml_dtypes.bfloat16
    x = np.asarray(x, dtype=np.float32)
    qkv_w = np.asarray(qkv_w, dtype=np.float32)
    wqkv = np.ascontiguousarray(qkv_w).astype(bf16)
    bqkv = np.ascontiguousarray(np.asarray(qkv_b, dtype=np.float32))
    wp = np.ascontiguousarray(np.asarray(proj_w, dtype=np.float32)).astype(bf16)
    bp = np.ascontiguousarray(np.asarray(proj_b, dtype=np.float32))
    in_maps = []
    for c in range(8):
        b, half = divmod(c, 2)
        xt = x[b].T  # [E, SEQ]
        if half == 0:
            xperm = xt
        else:
            xperm = np.concatenate([xt[:, QH:], xt[:, :QH]], axis=1)
        xperm = np.ascontiguousarray(xperm)
        in_maps.append(
            {
                "xt": xperm.astype(bf16),
                "wqkv": wqkv,
                "bqkv": bqkv,
                "wp": wp,
                "bv_bf": bqkv[2 * E : 3 * E].astype(bf16),
                "bp_bf": bp.astype(bf16),
            }
        )
    return in_maps


def assemble_out(results):
    out = np.empty((4, SEQ, E), dtype=np.float32)
    for c in range(8):
        b, half = divmod(c, 2)
        out[b, half * QH : (half + 1) * QH, :] = results[c]["out"]
    return out


def run(inputs, trace=False):
    """Run on 8 NeuronCores; returns (output, BassKernelResults)."""
    from concourse.bass_utils import run_bass_kernel_spmd

    nc = build_nc()
    in_maps = make_in_maps(**inputs)
    res = run_bass_kernel_spmd(nc, in_maps, core_ids=list(range(8)), trace=trace)
    return assemble_out(res.results), res


def kernel(x, qkv_w, qkv_b, proj_w, proj_b):
    out, _ = run(
        dict(x=x, qkv_w=qkv_w, qkv_b=qkv_b, proj_w=proj_w, proj_b=proj_b),
        trace=False,
    )
    return out


if __name__ == "__main__":
    rng = np.random.default_rng(0)
    x = rng.standard_normal((4, SEQ, E), dtype=np.float32)
    s = E ** -0.5
    inputs = dict(
        x=x,
        qkv_w=rng.standard_normal((E, 3 * E), dtype=np.float32) * s,
        qkv_b=rng.standard_normal((3 * E,), dtype=np.float32) * 0.02,
        proj_w=rng.standard_normal((E, E), dtype=np.float32) * s,
        proj_b=rng.standard_normal((E,), dtype=np.float32) * 0.02,
    )
    out = kernel(**inputs)
    print("out", out.shape, out.dtype, float(np.abs(out).mean()))


# revision 49
# speedup vs baseline: 1.0003x; 1.0003x over previous
"""Trainium2 Bass kernel for nn_Attention_9560597201123.

Full multi-head attention (B=4, N=2048, E=1024, H=16, D=64), f32 reference.

Sharding: 8 cores = (batch b in 0..4) x (sequence half in 0..2). Each core:
  - receives x[b].T (query-half columns first), full qkv/proj weights
  - computes k/v projections for the FULL batch-b sequence (2x redundant vs
    an exchange, but avoids slow 2-rank collectives entirely)
  - computes q projection + attention + output projection for its 1024
    query rows, returns y [1024, 1024]
Host assembles the 8 shards into [4, 2048, 1024].

Device layout notes:
  - scores are computed transposed (S^T: keys on partitions, queries free)
    so that P^T = exp(S^T) feeds the PV matmul directly (contraction = keys)
  - softmax normalizer: v is extended with a ones column (lhsT M=65), so
    the PV matmul's partition 64 accumulates the row sums for free
  - all TensorEngine matmuls run in bf16 (1 cycle/row); accumulation f32
  - chunked startup DMAs on parallel queues (xt per-ec on sync; weights on
    gpsimd/scalar) so the first matmuls start ~17us in
  - V projection interleaved with attention-0 qb=0 so the ScalarE exp
    stream (the per-phase floor: 256 x ~1.1us, one elem/cycle/lane) starts
    during the V phase
  - K/Q projections of pair p+1 emitted as small quanta between the S and
    PV matmuls of pair p's kc loop: TensorE's queue is strict FIFO, so the
    quanta must sit between S and PV to cover PV's exp-wait
  - k/q evictions on VectorE (tensor_scalar add) keeping ScalarE pure exp
  - softmax 1/Z via one partition-batched reciprocal_approx_fast per flush
    (rows gathered to partitions 0/32/64/96; free-dim cost is partition
    -count-independent) instead of 32 serial [1,512] reciprocals
  - tail: att(7,1) emitted before any output projection (FIFO head-of-line)
    and the projection split in halves so its first half overlaps the last
    exp stream and the last norm chain overlaps the first half
  - HAM warm-up: two short dummy-matmul bursts (gated on x chunks 0 and 4)
    keep the PE activity monitor at K=8/8 through the DMA-gated startup;
    without them the clock gate re-throttles to 1.2 GHz until ~36us
"""

import numpy as np
import ml_dtypes

P = 128
SEQ = 2048
QH = 1024  # queries per core
E = 1024
H = 16
NPAIR = 8  # head pairs
D = 64
KC = 16  # key chunks of 128
EC = 8  # e_in chunks of 128
SCALE = D ** -0.5  # 0.125

_NC = None


def build_nc():
    global _NC
    if _NC is not None:
        return _NC

    import concourse.bass as bass  # noqa: F401
    import concourse.mybir as mybir
    import concourse.tile as tile
    from concourse import bacc

    BF = mybir.dt.bfloat16
    F32 = mybir.dt.float32
    EXP = mybir.ActivationFunctionType.Exp
    ADD = mybir.AluOpType.add
    MULT = mybir.AluOpType.mult

    nc = bacc.Bacc("TRN2", target_bir_lowering=False, debug=False, num_devices=8)

    xt_d = nc.dram_tensor("xt", [E, SEQ], BF, kind="ExternalInput").ap()
    wqkv_d = nc.dram_tensor("wqkv", [E, 3 * E], BF, kind="ExternalInput").ap()
    bqkv_d = nc.dram_tensor("bqkv", [3 * E], F32, kind="ExternalInput").ap()
    wp_d = nc.dram_tensor("wp", [E, E], BF, kind="ExternalInput").ap()
    bv_bf_d = nc.dram_tensor("bv_bf", [E], BF, kind="ExternalInput").ap()
    bp_bf_d = nc.dram_tensor("bp_bf", [E], BF, kind="ExternalInput").ap()
    out_d = nc.dram_tensor("out", [QH, E], F32, kind="ExternalOutput").ap()

    wqkv_r = wqkv_d.rearrange("(o p) c -> p o c", p=P)

    with tile.TileContext(nc) as tc:
        with (
            tc.tile_pool(name="persist", bufs=1) as persist,
            tc.tile_pool(name="wstream", bufs=2) as wstream,
            tc.tile_pool(name="ptpool", bufs=4) as ptpool,
            tc.tile_pool(name="asbp", bufs=5) as asbp,
            tc.tile_pool(name="small", bufs=2) as small,
            tc.tile_pool(name="normp", bufs=1) as normp,
            tc.tile_pool(name="proj_ps", bufs=2, space="PSUM") as proj_ps,
            tc.tile_pool(name="acc_ps", bufs=2, space="PSUM") as acc_ps,
            tc.tile_pool(name="sc_ps", bufs=2, space="PSUM") as sc_ps,
        ):
            # ---- persistent tiles + input DMA ----
            # x lands in chunks on the sync queue so the first K/Q matmuls
            # start as early as possible
            xt_r = xt_d.rearrange("(o p) s -> p o s", p=P)
            xt3 = persist.tile([P, EC, SEQ], BF, tag="xt")
            for c in range(EC):
                nc.sync.dma_start(xt3[:, c : c + 1, :], xt_r[:, c : c + 1, :])
            xt = [xt3[:, ec, :] for ec in range(EC)]

            # HAM warm-up: ~4us of dummy matmuls gated only on xt chunk 0.
            # They fill the startup DMA wait and push the PE clock-gate to
            # K=8/8 before the real (DMA-gated, gappy) projection stream
            # begins; without this the PE re-throttles to 1.2 GHz until
            # ~36us and early matmuls run at 2x duration.
            warm = proj_ps.tile([P, 512], mybir.dt.float32, tag="ps512")
            for i in range(14):
                nc.tensor.matmul(
                    warm[:, 0:256],
                    lhsT=xt[0][:, 0:128],
                    rhs=xt[0][:, 0:256],
                    start=True,
                    stop=True,
                )
            # second burst gated on a mid-stream chunk keeps the PE busy
            # across the remaining x-chunk DMA window (a >3.4us gap would
            # re-throttle the clock gate). It shadows the chunk 4-7 arrival
            # window, which is DMA-bound anyway; traces show the 12-MM
            # version leaves phase-dependent 4/8 windows at ~18-38us.
            for i in range(26):
                nc.tensor.matmul(
                    warm[:, 0:256],
                    lhsT=xt[4][:, 0:128],
                    rhs=xt[4][:, 0:256],
                    start=True,
                    stop=True,
                )

            vx4 = []
            for sm in range(KC):
                t = persist.tile([P, H * 65], BF, tag=f"vx{sm}", name=f"vx{sm}").rearrange(
                    "p (h c) -> p h c", c=65
                )
                nc.vector.memset(t[:, :, 64], 1.0)
                vx4.append(t)

            kt = [persist.tile([P, SEQ], BF, tag=f"kt{p}", name=f"kt{p}") for p in range(NPAIR)]
            qt = [persist.tile([P, QH], BF, tag=f"qt{p}", name=f"qt{p}") for p in range(NPAIR)]
            aT = [persist.tile([P, QH], BF, tag=f"aT{p}", name=f"aT{p}") for p in range(NPAIR)]

            bq_k = persist.tile([P, EC], F32, tag="bq_k")
            bq_q = persist.tile([P, EC], F32, tag="bq_q")

            bv_row = persist.tile([1, E], BF, tag="bv_row")
            nc.gpsimd.dma_start(bv_row[:], bv_bf_d[None])
            bv_bc = persist.tile([P, E], BF, tag="bv_bc")
            nc.gpsimd.partition_broadcast(bv_bc[:], bv_row[:])

            pb_row = persist.tile([1, E], BF, tag="pb_row")
            nc.gpsimd.dma_start(pb_row[:], bp_bf_d[None])
            pb_bc = persist.tile([P, E], BF, tag="pb_bc")
            nc.gpsimd.partition_broadcast(pb_bc[:], pb_row[:])

            pw = None  # allocated after the wvpool frees its SBUF space

            # ---- emission helpers ----

            def kq_quanta(p):
                """Return a list of emission closures for pair p's K/Q
                projections: 6 output blocks x 2 half-blocks (4 matmuls
                each). Interleaved into the attention kc loop so the exp
                stream never starves behind a long projection burst."""
                wq = wstream.tile([P, EC, P], BF, tag="wq")
                nc.gpsimd.dma_start(wq[:], wqkv_r[:, :, p * P : (p + 1) * P])
                wk = wstream.tile([P, EC, P], BF, tag="wk")
                nc.gpsimd.dma_start(wk[:], wqkv_r[:, :, E + p * P : E + (p + 1) * P])
                blocks = [("q", s) for s in range(2)] + [("k", s) for s in range(4)]
                quanta = []
                state = {}

                def mk(which, s, half):
                    w = wq if which == "q" else wk
                    dst = qt[p] if which == "q" else kt[p]
                    bias = bq_q if which == "q" else bq_k

                    def q_():
                        if half == 0:
                            state[(which, s)] = proj_ps.tile(
                                [P, 512],
                                mybir.dt.float32,
                                tag="ps512",
                                name=f"ps_{p}_{which}{s}",
                            )
                        ps = state[(which, s)]
                        for ec in range(4 * half, 4 * half + 4):
                            nc.tensor.matmul(
                                ps[:],
                                lhsT=w[:, ec, :],
                                rhs=xt[ec][:, s * 512 : (s + 1) * 512],
                                start=(ec == 0),
                                stop=(ec == EC - 1),
                            )
                        if half == 1:
                            nc.vector.tensor_scalar(
                                out=dst[:, s * 512 : (s + 1) * 512],
                                in0=ps[:],
                                scalar1=bias[:, p : p + 1],
                                scalar2=None,
                                op0=ADD,
                            )

                    return q_

                for which, s in blocks:
                    quanta.append(mk(which, s, 0))
                    quanta.append(mk(which, s, 1))
                return quanta

            def emit_kq(p):
                for q_ in kq_quanta(p):
                    q_()

            def emit_v_chunk(wv_tiles, sm):
                # both vc halves of v for key chunk sm (completes vx4[sm])
                for vc in range(2):
                    ps = proj_ps.tile([P, 512], mybir.dt.float32, tag="ps512")
                    for ec in range(EC):
                        nc.tensor.matmul(
                            ps[:],
                            lhsT=xt[ec][:, sm * P : (sm + 1) * P],
                            rhs=wv_tiles[vc][:, ec, :],
                            start=(ec == 0),
                            stop=(ec == EC - 1),
                        )
                    nc.vector.tensor_tensor(
                        out=vx4[sm][:, vc * 8 : (vc + 1) * 8, 0:64],
                        in0=ps[:].rearrange("p (h c) -> p h c", c=64),
                        in1=bv_bc[:, vc * 512 : (vc + 1) * 512].rearrange(
                            "p (h c) -> p h c", c=64
                        ),
                        op=ADD,
                    )

            pending_rows = []  # (asb, hh, p, qsl) awaiting normalize

            def emit_S(p, qb, kc):
                qsl = slice(qb * 512, (qb + 1) * 512)
                sc = sc_ps.tile([P, 1024], mybir.dt.float32, tag="sc")
                nc.tensor.matmul(
                    sc[:, 0:512],
                    lhsT=kt[p][0:64, kc * P : (kc + 1) * P],
                    rhs=qt[p][0:64, qsl],
                )
                nc.tensor.matmul(
                    sc[:, 512:1024],
                    lhsT=kt[p][64:P, kc * P : (kc + 1) * P],
                    rhs=qt[p][64:P, qsl],
                )
                pt = ptpool.tile([P, 1024], BF, tag="pt")
                nc.scalar.activation(out=pt[:], in_=sc[:], func=EXP, scale=SCALE)
                return pt

            def emit_PV(p, kc, pt, accA, accB):
                nc.tensor.matmul(
                    accA[:],
                    lhsT=vx4[kc][:, 2 * p, :],
                    rhs=pt[:, 0:512],
                    start=(kc == 0),
                    stop=(kc == KC - 1),
                )
                nc.tensor.matmul(
                    accB[:],
                    lhsT=vx4[kc][:, 2 * p + 1, :],
                    rhs=pt[:, 512:1024],
                    start=(kc == 0),
                    stop=(kc == KC - 1),
                )

            def emit_att_kc(p, qb, kc, accA, accB):
                pt = emit_S(p, qb, kc)
                emit_PV(p, kc, pt, accA, accB)

            def evict_acc(p, qb, accA, accB):
                qsl = slice(qb * 512, (qb + 1) * 512)
                for hh, acc in ((0, accA), (1, accB)):
                    asb = asbp.tile([65, 512], F32, tag="asb")
                    nc.vector.tensor_copy(out=asb[:], in_=acc[:])
                    pending_rows.append((asb, hh, p, qsl))

            def emit_att_qb(p, qb, fill=None):
                """Attention for (p, qb) in 2-kc groups: both S-pairs (64-row
                array config) back to back, then 4 PV matmuls (128-row
                config), then fill quanta. Halves the tile-config switches
                whose LDWEIGHTS cannot be pulled ahead. `fill` is a mutable
                list of emission closures (projection quanta) drained two per
                group so TensorE fill-work stays fine-grained."""
                accA = acc_ps.tile([65, 512], mybir.dt.float32, tag="acc")
                accB = acc_ps.tile([65, 512], mybir.dt.float32, tag="acc")
                for g in range(KC // 2):
                    pt0 = emit_S(p, qb, 2 * g)
                    pt1 = emit_S(p, qb, 2 * g + 1)
                    # fill BETWEEN S and PV: TensorE's queue is strict FIFO,
                    # so work placed after PV cannot cover PV's exp-wait.
                    # One quantum per group spreads the 12 quanta across both
                    # qb windows, feeding qb1's otherwise exp-gated groups.
                    if fill:
                        fill.pop(0)()
                    emit_PV(p, 2 * g, pt0, accA, accB)
                    emit_PV(p, 2 * g + 1, pt1, accA, accB)
                evict_acc(p, qb, accA, accB)
                while fill:
                    fill.pop(0)()

            def flush_norm():
                """Normalize all pending rows (max 4) with one
                partition-batched reciprocal (free-dim cost is independent of
                partition count). Rows are spaced 32 partitions apart because
                DVE access patterns must start on a 32-aligned partition."""
                rows = list(pending_rows)
                pending_rows.clear()
                if not rows:
                    return
                assert len(rows) <= 4
                # hh=1 first: its chain ends in a DMA that then overlaps the
                # hh=0 multiplies
                rows.sort(key=lambda r: -r[1])
                zg = normp.tile([97, 512], F32, tag="zg")
                for i, (asb, hh, p, qsl) in enumerate(rows):
                    nc.sync.dma_start(zg[32 * i : 32 * i + 1, :], asb[64:65, :])
                zr = normp.tile([97, 512], F32, tag="zr")
                nc.vector.reciprocal_approx_fast(out=zr[:], in_=zg[:])
                for i, (asb, hh, p, qsl) in enumerate(rows):
                    rs = small.tile([1, 512], BF, tag="rs")
                    nc.vector.tensor_copy(out=rs[:], in_=zr[32 * i : 32 * i + 1, :])
                    R = small.tile([64, 512], BF, tag="R")
                    nc.gpsimd.partition_broadcast(R[:], rs[:])
                    if hh == 0:
                        nc.vector.tensor_tensor(
                            out=aT[p][0:64, qsl], in0=asb[0:64, :], in1=R[:], op=MULT
                        )
                    else:
                        tmpb = small.tile([64, 512], BF, tag="tmpb")
                        nc.vector.tensor_tensor(
                            out=tmpb[:], in0=asb[0:64, :], in1=R[:], op=MULT
                        )
                        nc.sync.dma_start(aT[p][64:P, qsl], tmpb[:])

            def emit_proj(pw, qc_list, dma_eng=None):
                for qc in qc_list:
                    for ncol in range(2):
                        nsl = slice(ncol * 512, (ncol + 1) * 512)
                        yps = proj_ps.tile([P, 512], mybir.dt.float32, tag="ps512")
                        for p in range(NPAIR):
                            nc.tensor.matmul(
                                yps[:],
                                lhsT=aT[p][:, qc * P : (qc + 1) * P],
                                rhs=pw[:, p, nsl],
                                start=(p == 0),
                                stop=(p == NPAIR - 1),
                            )
                        ysb = small.tile([P, 512], F32, tag="ysb")
                        nc.vector.tensor_tensor(
                            out=ysb[:], in0=yps[:], in1=pb_bc[:, nsl], op=ADD
                        )
                        (dma_eng or nc.sync).dma_start(
                            out_d[qc * P : (qc + 1) * P, nsl], ysb[:]
                        )

            # ---- schedule ----
            # pair 0 K/Q first; V interleaved with attention-0 qb=0 so the
            # exp stream starts during the V projection; then the usual
            # software pipeline (K/Q of pair p+1 ahead of attention p).
            q0 = kq_quanta(0)  # wq0/wk0 DMAs land first on the gpsimd queue
            nc.gpsimd.dma_start(bq_k[:], bqkv_d[E : 2 * E].rearrange("(o p) -> p o", p=P))
            nc.gpsimd.dma_start(bq_q[:], bqkv_d[0:E].rearrange("(o p) -> p o", p=P))
            for q_ in q0:
                q_()
            with tc.tile_pool(name="wvpool", bufs=2) as wvpool:
                wv_tiles = []
                for vc in range(2):
                    wv = wvpool.tile([P, EC, 512], BF, tag="wv")
                    nc.scalar.dma_start(
                        wv[:], wqkv_r[:, :, 2 * E + vc * 512 : 2 * E + (vc + 1) * 512]
                    )
                    wv_tiles.append(wv)
                accA = acc_ps.tile([65, 512], mybir.dt.float32, tag="acc")
                accB = acc_ps.tile([65, 512], mybir.dt.float32, tag="acc")
                for sm in range(KC):
                    emit_v_chunk(wv_tiles, sm)
                    emit_att_kc(0, 0, sm, accA, accB)
                evict_acc(0, 0, accA, accB)

            with tc.tile_pool(name="pwpool", bufs=1) as pwpool:
                # proj weights on the scalar DMA queue (after wv) so they
                # don't contend with the startup xt stream on sync
                pw = pwpool.tile([P, NPAIR, E], BF, tag="pw")
                nc.scalar.dma_start(pw[:], wp_d.rearrange("(o p) c -> p o c", p=P))

                fill = kq_quanta(1)
                emit_att_qb(0, 1, fill)
                for p in range(1, NPAIR - 1):
                    flush_norm()
                    fill = kq_quanta(p + 1)
                    emit_att_qb(p, 0, fill)
                    emit_att_qb(p, 1, fill)
                # last pair: att(7,1) is emitted BEFORE any proj so its exp
                # stream follows att(7,0) without a stall (TensorE queue is
                # strict FIFO — proj emitted earlier would head-of-line block
                # it behind the norm chain). proj qc 0-3 needs only qb=0
                # norms, which complete while att(7,1) runs; the (7,1) norm
                # chain then overlaps proj qc 0-3.
                flush_norm()
                emit_att_qb(NPAIR - 1, 0)
                flush_norm()
                emit_att_qb(NPAIR - 1, 1)
                emit_proj(pw, range(0, 4), dma_eng=nc.scalar)
                flush_norm()
                emit_proj(pw, range(4, 8))

    nc.finalize()
    _NC = nc
    return nc


def make_in_maps(x, qkv_w, qkv_b, proj_w, proj_b):
    bf16 = ml_dtypes.bfloat16
    x = np.asarray(x, dtype=np.float32)
    qkv_w = np.asarray(qkv_w, dtype=np.float32)
    wqkv = np.ascontiguousarray(qkv_w).astype(bf16)
    bqkv = np.ascontiguousarray(np.asarray(qkv_b, dtype=np.float32))
    wp = np.ascontiguousarray(np.asarray(proj_w, dtype=np.float32)).astype(bf16)
    bp = np.ascontiguousarray(np.asarray(proj_b, dtype=np.float32))
    in_maps = []
    for c in range(8):
        b, half = divmod(c, 2)
        xt = x[b].T  # [E, SEQ]
        if half == 0:
            xperm = xt
        else:
            xperm = np.concatenate([xt[:, QH:], xt[:, :QH]], axis=1)
        xperm = np.ascontiguousarray(xperm)
        in_maps.append(
            {
                "xt": xperm.astype(bf16),
                "wqkv": wqkv,
                "bqkv": bqkv,
                "wp": wp,
                "bv_bf": bqkv[2 * E : 3 * E].astype(bf16),
                "bp_bf": bp.astype(bf16),
            }
        )
    return in_maps


def assemble_out(results):
    out = np.empty((4, SEQ, E), dtype=np.float32)
    for c in range(8):
        b, half = divmod(c, 2)
        out[b, half * QH : (half + 1) * QH, :] = results[c]["out"]
    return out


def run(inputs, trace=False):
    """Run on 8 NeuronCores; returns (output, BassKernelResults)."""
    from concourse.bass_utils import run_bass_kernel_spmd

    nc = build_nc()
    in_maps = make_in_maps(**inputs)
    res = run_bass_kernel_spmd(nc, in_maps, core_ids=list(range(8)), trace=trace)
    return assemble_out(res.results), res


def kernel(x, qkv_w, qkv_b, proj_w, proj_b):
    out, _ = run(
        dict(x=x, qkv_w=qkv_w, qkv_b=qkv_b, proj_w=proj_w, proj_b=proj_b),
        trace=False,
    )
    return out


if __name__ == "__main__":
    rng = np.random.default_rng(0)
    x = rng.standard_normal((4, SEQ, E), dtype=np.float32)
    s = E ** -0.5
    inputs = dict(
        x=x,
        qkv_w=rng.standard_normal((E, 3 * E), dtype=np.float32) * s,
        qkv_b=rng.standard_normal((3 * E,), dtype=np.float32) * 0.02,
        proj_w=rng.standard_normal((E, E), dtype=np.float32) * s,
        proj_b=rng.standard_normal((E,), dtype=np.float32) * 0.02,
    )
    out = kernel(**inputs)
    print("out", out.shape, out.dtype, float(np.abs(out).mean()))
